# revision 7
# baseline (speedup 1.0000x reference)
"""Trainium2 Bass kernel for NeuroplasticLlama block-sparse adapter (moe_routing).

Contract: kernel(**inputs) takes FULL unsharded inputs (as produced by
setup_inputs) and returns the FULL [4, 4096, 4096] float32 output.

Strategy (data/sequence parallel over 8 cores, 2048 tokens each):
  - Each core's 2048 contiguous tokens belong to exactly one batch, so the
    task embedding contributes only per-core constant bias vectors
    (te @ A folded into the z bias, te @ Wp folded into the coords bias)
    -- h = x + te is never materialized.
  - Routing is rank-3: scores s[t,n] = coords[t]·mu_n - |mu_n|^2/2 with
    coords = x @ Wp + (te @ Wp + bp).  coords is a K=4096 fp8-DoubleRow
    matmul with M=3; scores are then tiny K=4 matmuls producing s token-major
    [t, n] directly (no score transposes).
  - top-3 selection via threshold = 3rd max (MAX8 + mask), gates
    g = exp(s - max) * (s >= thr3) / sum(...)  (DVE chain).
  - z (all 512 block-rank pairs) = x @ A_all, dense fp8 DoubleRow.
    zg = (z/8) * expand4(g) in fp8; delta = block-diag(8*Bm) matmuls run
    4-way ROW-TILED (K=32 strips at partition bases 0/32/64/96 with
    zero-padded weights) so 4 hidden-chunk matmuls stream concurrently in
    one PE pass.  Delta rounds are interleaved between z/coords
    half-groups so the PE never waits on PSUM drains.
  - y = x + delta: psum drained into the x tiles by a balanced mix of
    DVE/Pool direct-psum adds and ACT-copy + bf16 adds, then stored per
    finished [128, 4096] slice (1 MB DMAs).
  - I/O is bf16 (host converts); x also ships as a packed fp8 copy for the
    PE.  Large DMAs: 2 MB fp8 + 4 MB bf16 per macrotile.
"""

import sys

if "/opt/trn_rl_repo" not in sys.path:
    sys.path.insert(0, "/opt/trn_rl_repo")

import numpy as np
import ml_dtypes

H = 4096
NB = 128
BLK = 32
R = 4
B = 4
S = 4096
NCORES = 8
TPC = (B * S) // NCORES  # tokens per core = 2048
T = 512                  # tokens per macrotile
NMT = TPC // T           # 4 macrotiles per core
NKT = H // 128           # 32 k-tiles over the hidden dim
MTW = 8 * 2048           # columns per macrotile in the packed layout
ZSC = 0.125              # z is scaled by 1/8 before fp8, Bm by 8

TRACE = False            # set by test.py for profiling runs
TRACE_DIR = None
LAST_RESULT = None       # BassKernelResults of the last run

_COMPILED = None


def _build():
    import concourse.bacc as bacc
    import concourse.tile as tile
    from concourse import mybir, masks

    f32 = mybir.dt.float32
    bf16 = mybir.dt.bfloat16
    f8 = mybir.dt.float8e4
    AF = mybir.ActivationFunctionType
    AL = mybir.AluOpType
    DR = mybir.MatmulPerfMode.DoubleRow

    nc = bacc.Bacc("TRN2", target_bir_lowering=False, debug=False,
                   num_devices=NCORES)

    xt_d = nc.dram_tensor("xt", [128, NMT * MTW], bf16, kind="ExternalInput")
    xb_d = nc.dram_tensor("xb8", [128, NMT * MTW], f8, kind="ExternalInput")
    az_d = nc.dram_tensor("az", [128, 4 * NKT * 128], f8, kind="ExternalInput")
    wp_d = nc.dram_tensor("wp", [128, NKT * 16], f8, kind="ExternalInput")
    bpk_d = nc.dram_tensor("bpk2", [128, 1024], f8, kind="ExternalInput")
    e_d = nc.dram_tensor("e", [128, 512], bf16, kind="ExternalInput")
    cen_d = nc.dram_tensor("cen", [4, 128], bf16, kind="ExternalInput")
    bias_d = nc.dram_tensor("bias", [128, 5], f32, kind="ExternalInput")
    yt_d = nc.dram_tensor("yt", [128, NMT * MTW], bf16, kind="ExternalOutput")

    xt_ap = xt_d.ap()
    xb_ap = xb_d.ap()
    yt_ap = yt_d.ap()

    with tile.TileContext(nc) as tc:
        from contextlib import ExitStack
        with ExitStack() as ctx:
            cpool = ctx.enter_context(tc.tile_pool(name="consts", bufs=1))
            xpool = ctx.enter_context(tc.tile_pool(name="xg", bufs=3))
            xbpool = ctx.enter_context(tc.tile_pool(name="xb", bufs=2))
            zpool = ctx.enter_context(tc.tile_pool(name="zb", bufs=6))
            gpool = ctx.enter_context(tc.tile_pool(name="gate", bufs=3))
            spool = ctx.enter_context(tc.tile_pool(name="scal", bufs=4))
            pp = ctx.enter_context(tc.tile_pool(name="ps", bufs=2, space="PSUM"))

            NTS = T // 128  # token sub-tiles per macrotile

            # ---- x tiles; mt0 fp8 halves issued before heavy consts ----
            XB, XG, XGV = [], [], []
            for mt in range(NMT):
                xb = xbpool.tile([128, MTW], f8, name="xb", tag="xb")
                xg = xpool.tile([128, MTW], bf16, name="xg", tag="xg")
                XB.append(xb)
                XG.append(xg)
                XGV.append(xg[:].rearrange(
                    "p (g twoc r t) -> p g twoc r t", g=8, twoc=2, r=2))
            nc.sync.dma_start(XB[0][:, 0:MTW // 2], xb_ap[:, 0:MTW // 2])
            nc.sync.dma_start(XB[0][:, MTW // 2:], xb_ap[:, MTW // 2:MTW])

            # ---- consts on the gpsimd (SWDGE) queue, smallest first ----
            wp8 = cpool.tile([128, NKT * 16], f8, name="wp8", tag="wp8")
            nc.gpsimd.dma_start(wp8[:], wp_d.ap()[:])
            cen = cpool.tile([4, 128], bf16, name="cen", tag="cen")
            nc.gpsimd.dma_start(cen[:], cen_d.ap()[:])
            bias = cpool.tile([128, 5], f32, name="bias", tag="bias")
            nc.gpsimd.dma_start(bias[:], bias_d.ap()[:])
            esb = cpool.tile([128, 512], bf16, name="esb", tag="esb")
            nc.gpsimd.dma_start(esb[:], e_d.ap()[:])
            bpk2 = cpool.tile([128, 1024], f8, name="bpk2", tag="bpk2")
            nc.gpsimd.dma_start(bpk2[:], bpk_d.ap()[:])
            az = []
            for q in range(4):
                t_az = cpool.tile([128, NKT * 128], f8, name=f"az{q}",
                                  tag=f"az{q}")
                nc.gpsimd.dma_start(
                    t_az[:], az_d.ap()[:, q * NKT * 128:(q + 1) * NKT * 128])
                az.append(t_az)
            identf = cpool.tile([128, 128], f32, name="identf", tag="identf")
            masks.make_identity(nc, identf[:])

            # ---- remaining x loads, in need order on the sync HWDGE ring ----
            nc.sync.dma_start(XG[0][:], xt_ap[:, 0:MTW])
            for mt in range(1, NMT):
                nc.sync.dma_start(XB[mt][:],
                                  xb_ap[:, mt * MTW:(mt + 1) * MTW])
                nc.sync.dma_start(XG[mt][:],
                                  xt_ap[:, mt * MTW:(mt + 1) * MTW])

            ZB = [[None] * 4 for _ in range(NMT)]
            ZG = [[None] * 4 for _ in range(NMT)]
            GT = [None] * NMT
            GGs = [None] * NMT
            CSB = [None] * NMT

            # ---------------- stage helpers ----------------
            def coords_half(mt, cp, lo, hi):
                for k2 in range(lo, hi):
                    nc.tensor.matmul(
                        cp[:],
                        wp8[:, k2 * 32:(k2 + 1) * 32]
                        .rearrange("p (two m) -> p two m", two=2),
                        XB[mt][:, k2 * 2 * T:(k2 + 1) * 2 * T]
                        .rearrange("p (two t) -> p two t", two=2),
                        start=(k2 == 0), stop=(k2 == NKT // 2 - 1),
                        perf_mode=DR,
                    )

            def csb_stage(mt, cp):
                csb = gpool.tile([4, T], bf16, name="csb", tag="csb", bufs=2)
                nc.gpsimd.memset(csb[:], 1.0)
                nc.scalar.activation(csb[0:3, :], cp[0:3, :], AF.Identity,
                                     bias=bias[0:3, 4:5], scale=1.0)
                CSB[mt] = csb

            def scores_stage(mt):
                csb = CSB[mt]
                sp = pp.tile([128, 4 * 128], f32, space="PSUM", name="sp",
                             tag="sp", bufs=1)
                for ts in range(NTS):
                    nc.tensor.matmul(sp[:, ts * 128:(ts + 1) * 128],
                                     csb[:, ts * 128:(ts + 1) * 128],
                                     cen[:], start=True, stop=True)
                ggs = []
                for ts in range(NTS):
                    ssl = sp[:, ts * 128:(ts + 1) * 128]
                    m8 = spool.tile([128, 8], f32, name="m8", tag="m8")
                    nc.vector.max(m8[:], ssl)
                    nr1 = spool.tile([128, 1], f32, name="nr1", tag="nr1")
                    nc.vector.tensor_scalar_mul(nr1[:], m8[:, 0:1], -1.0)
                    ex = gpool.tile([128, 128], f32, name="ex", tag="ex")
                    nc.scalar.activation(ex[:], ssl, AF.Exp, bias=nr1[:],
                                         scale=1.0)
                    em = gpool.tile([128, 128], f32, name="em", tag="em")
                    zs = spool.tile([128, 1], f32, name="zs", tag="zs")
                    nc.vector.scalar_tensor_tensor(em[:], ssl, m8[:, 2:3],
                                                   ex[:], AL.is_ge, AL.mult,
                                                   accum_out=zs[:])
                    rz = spool.tile([128, 1], f32, name="rz", tag="rz")
                    nc.vector.reciprocal(rz[:], zs[:])
                    gg = gpool.tile([128, 128], f32, name="gg", tag="gg",
                                    bufs=NTS + 1)
                    nc.gpsimd.tensor_scalar_mul(gg[:], em[:], rz[:])
                    ggs.append(gg)
                GGs[mt] = ggs

            def z_half(mt, q, zp, lo, hi):
                for k2 in range(lo, hi):
                    nc.tensor.matmul(
                        zp[:],
                        az[q][:, k2 * 256:(k2 + 1) * 256]
                        .rearrange("p (two m) -> p two m", two=2),
                        XB[mt][:, k2 * 2 * T:(k2 + 1) * 2 * T]
                        .rearrange("p (two t) -> p two t", two=2),
                        start=(k2 == 0), stop=(k2 == NKT // 2 - 1),
                        perf_mode=DR,
                    )

            def zb_stage(mt, q, zp):
                zb = zpool.tile([128, T], bf16, name="zb", tag="zb")
                nc.scalar.activation(zb[:], zp[:], AF.Identity,
                                     bias=bias[:, q:q + 1], scale=ZSC)
                ZB[mt][q] = zb

            def transpose_stage(mt):
                gt_sb = gpool.tile([128, T], bf16, name="gt_sb", tag="gt_sb",
                                   bufs=2)
                g_ps = pp.tile([128, 4 * 128], f32, space="PSUM", name="g_ps",
                               tag="sp", bufs=1)
                for ts in range(NTS):
                    nc.tensor.transpose(g_ps[:, ts * 128:(ts + 1) * 128],
                                        GGs[mt][ts][:], identf[:])
                nc.scalar.copy(gt_sb[:], g_ps[:])
                GT[mt] = gt_sb

            def expand_stage(mt, q):
                gx = pp.tile([128, T], f32, space="PSUM", name="gx",
                             tag="zp", bufs=2)
                nc.tensor.matmul(gx[:],
                                 esb[:, q * 128:(q + 1) * 128],
                                 GT[mt][:],
                                 start=True, stop=True)
                zg = zpool.tile([128, T], f8, name="zg", tag="zg")
                nc.vector.tensor_mul(zg[:], ZB[mt][q][:], gx[:])
                ZG[mt][q] = zg

            drain_ctr = [0]

            def dq_round(mt, q, r, last=False):
                zg = ZG[mt][q]
                dp = pp.tile([128, 2048], f32, space="PSUM", name="dp",
                             tag="dp", bufs=1)
                for s in range(4):
                    nc.tensor.matmul(
                        dp[:, s * T:(s + 1) * T],
                        bpk2[32 * s:32 * s + 32,
                             (q * 2 + r) * 128:(q * 2 + r + 1) * 128],
                        zg[32 * s:32 * s + 32, :],
                        start=True, stop=True,
                        tile_position=(32 * s, 0))
                i = drain_ctr[0]
                drain_ctr[0] += 1
                # GPSIMD cannot read PSUM: psum is drained by DVE (direct
                # [128,2048] add) or ACT (copy to bf16, Pool adds it).
                dst4 = XGV[mt][:, 2 * q:2 * q + 2, :, r, :]
                if last:
                    # split across DVE + ACT for low dp-recycle latency
                    d1 = XGV[mt][:, 2 * q, :, r, :]
                    s1 = dp[:, 0:1024].rearrange("p (two t) -> p two t",
                                                 two=2)
                    nc.vector.tensor_add(d1, d1, s1)
                    dsb = zpool.tile([128, 1024], bf16, name="dsbl",
                                     tag="dsbl", bufs=2)
                    nc.scalar.copy(dsb[:], dp[:, 1024:2048])
                    d2 = XGV[mt][:, 2 * q + 1, :, r, :]
                    nc.gpsimd.tensor_add(
                        d2, d2,
                        dsb[:].rearrange("p (two t) -> p two t", two=2))
                elif i % 2 == 0:
                    src4 = dp[:].rearrange("p (g twoc t) -> p g twoc t",
                                           g=2, twoc=2)
                    nc.vector.tensor_add(dst4, dst4, src4)
                else:
                    dsb = zpool.tile([128, 2048], bf16, name="dsb",
                                     tag="dsb", bufs=4)
                    nc.scalar.copy(dsb[:], dp[:])
                    nc.gpsimd.tensor_add(
                        dst4, dst4,
                        dsb[:].rearrange("p (g twoc t) -> p g twoc t",
                                         g=2, twoc=2))

            def dq_store(mt, q, last=False):
                eng = nc.sync if last else nc.gpsimd
                eng.dma_start(
                    yt_ap[:, mt * MTW + q * 4096:mt * MTW + (q + 1) * 4096],
                    XG[mt][:, q * 4096:(q + 1) * 4096])

            # ---------------- interleaved emission ----------------
            for mt in range(NMT):
                p = mt - 1
                cp = pp.tile([16, T], f32, space="PSUM", name="cp", tag="cp",
                             bufs=1)
                coords_half(mt, cp, 0, NKT // 4)
                if p >= 0:
                    dq_round(p, 2, 0)
                coords_half(mt, cp, NKT // 4, NKT // 2)
                csb_stage(mt, cp)
                if p >= 0:
                    dq_round(p, 2, 1)
                    dq_store(p, 2)
                scores_stage(mt)

                zp0 = pp.tile([128, T], f32, space="PSUM", name="zp",
                              tag="zp")
                z_half(mt, 0, zp0, 0, NKT // 4)
                if p >= 0:
                    dq_round(p, 3, 0)
                z_half(mt, 0, zp0, NKT // 4, NKT // 2)
                if p >= 0:
                    dq_round(p, 3, 1)
                    dq_store(p, 3)
                zb_stage(mt, 0, zp0)

                zp1 = pp.tile([128, T], f32, space="PSUM", name="zp",
                              tag="zp")
                z_half(mt, 1, zp1, 0, NKT // 4)
                transpose_stage(mt)
                z_half(mt, 1, zp1, NKT // 4, NKT // 2)
                zb_stage(mt, 1, zp1)

                expand_stage(mt, 0)

                zp2 = pp.tile([128, T], f32, space="PSUM", name="zp",
                              tag="zp")
                z_half(mt, 2, zp2, 0, NKT // 4)
                dq_round(mt, 0, 0)
                expand_stage(mt, 1)
                z_half(mt, 2, zp2, NKT // 4, NKT // 2)
                dq_round(mt, 0, 1)
                dq_store(mt, 0)
                zb_stage(mt, 2, zp2)

                expand_stage(mt, 2)

                zp3 = pp.tile([128, T], f32, space="PSUM", name="zp",
                              tag="zp")
                z_half(mt, 3, zp3, 0, NKT // 4)
                dq_round(mt, 1, 0)
                z_half(mt, 3, zp3, NKT // 4, NKT // 2)
                dq_round(mt, 1, 1)
                dq_store(mt, 1)
                zb_stage(mt, 3, zp3)

                expand_stage(mt, 3)

            # tail: last macrotile's q2/q3 deltas
            m3 = NMT - 1
            dq_round(m3, 2, 0, last=True)
            dq_round(m3, 2, 1, last=True)
            dq_store(m3, 2)
            dq_round(m3, 3, 0, last=True)
            dq_round(m3, 3, 1, last=True)
            dq_store(m3, 3, last=True)

    nc.compile()
    return nc


def _prep_consts(task_emb, task_ids, Wp, bp, centers, A, Bm, adapter_scale):
    scale = float(np.asarray(adapter_scale))
    A_all = np.ascontiguousarray(
        A.transpose(1, 0, 2).reshape(H, NB * R).astype(np.float32))

    # az: [p, q, hc, m] = A_all[hc*128+p, q*128+m], fp8 e4m3 (DoubleRow pairs
    # of consecutive hc become the [K,2,M] interleave)
    az = (A_all.reshape(NKT, 128, 4, 128).transpose(1, 2, 0, 3)
          .reshape(128, 4 * NKT * 128).astype(ml_dtypes.float8_e4m3))
    az = np.ascontiguousarray(az)

    # wp8: [p, k2, two, c] = Wp[(2*k2+two)*128+p, c] (c padded 3->16:
    # fp8 DoubleRow LDWEIGHTS requires the pair stride to be 16B-aligned), fp8
    wpp = np.zeros((H, 16), np.float32)
    wpp[:, 0:3] = Wp.astype(np.float32)
    wp8 = (wpp.reshape(NKT // 2, 2, 128, 16)
           .transpose(2, 0, 1, 3).reshape(128, NKT * 16)
           .astype(ml_dtypes.float8_e4m3))
    wp8 = np.ascontiguousarray(wp8)

    # block-diag up-projection, row-tiled layout (x8 to keep fp8 in normal
    # range; z is /8).  bpk2[32*s + 16*r + mblk*4 + rr,
    #                       (q*2+r)*128 + mblk*32 + c] = Bm[n, rr, c]*scale*8
    # with chunk j = 2*s + r of q, n = (8*q + j)*4 + mblk; other rows zero.
    bpk2 = np.zeros((128, 1024), np.float32)
    for q in range(4):
        for s in range(4):
            for r in range(2):
                j = 2 * s + r
                hc = 8 * q + j
                for mblk in range(4):
                    n = hc * 4 + mblk
                    for rr in range(R):
                        row = 32 * s + 16 * r + mblk * 4 + rr
                        col = (q * 2 + r) * 128 + mblk * 32
                        bpk2[row, col:col + 32] = Bm[n, rr, :] * scale * 8.0
    bpk2 = bpk2.astype(ml_dtypes.float8_e4m3)

    e_np = (np.arange(128)[:, None] == (np.arange(512)[None, :] // 4)) \
        .astype(ml_dtypes.bfloat16)

    # cen_aug: rows 0-2 = centers.T, row 3 = -|mu|^2/2
    cen = np.zeros((4, 128), np.float32)
    cen[0:3] = centers.T
    cen[3] = -0.5 * (centers ** 2).sum(-1)
    cen = np.ascontiguousarray(cen.astype(ml_dtypes.bfloat16))

    biases = []
    for c in range(NCORES):
        te = task_emb[int(np.asarray(task_ids)[c // 2])].astype(np.float32)
        b5 = np.zeros((128, 5), np.float32)
        zoff = (te @ A_all) * ZSC                                # [512]
        for q in range(4):
            b5[:, q] = zoff[q * 128:(q + 1) * 128]
        b5[0:3, 4] = te @ Wp + bp                                # coords bias
        biases.append(np.ascontiguousarray(b5))
    return az, wp8, bpk2, e_np, cen, biases


def _pack_x(xc):
    # [TPC, H] f32 -> [128, NMT*8*2048] in kernel tile order (bf16 + fp8)
    t = np.ascontiguousarray(xc.reshape(NMT, T, 8, 4, 128)
                             .transpose(4, 0, 2, 3, 1)
                             .reshape(128, NMT * MTW))
    return t.astype(ml_dtypes.bfloat16), t.astype(ml_dtypes.float8_e4m3)


def _unpack_y(yt):
    # [128, NMT*8*2048] bf16 -> [TPC, H] f32
    t = yt.reshape(128, NMT, 8, 4, T).transpose(1, 4, 2, 3, 0)
    return t.reshape(TPC, H).astype(np.float32)


def kernel(x, task_ids, task_emb, Wp, bp, centers, A, Bm, adapter_scale):
    global _COMPILED, LAST_RESULT
    from concourse import bass_utils

    x = np.asarray(x, dtype=np.float32)
    task_ids = np.asarray(task_ids)
    task_emb = np.asarray(task_emb, dtype=np.float32)
    Wp = np.asarray(Wp, dtype=np.float32)
    bp = np.asarray(bp, dtype=np.float32)
    centers = np.asarray(centers, dtype=np.float32)
    A = np.asarray(A, dtype=np.float32)
    Bm = np.asarray(Bm, dtype=np.float32)

    if _COMPILED is None:
        _COMPILED = _build()
    nc = _COMPILED

    az, wp8, bpk2, e_np, cen, biases = _prep_consts(
        task_emb, task_ids, Wp, bp, centers, A, Bm, adapter_scale)

    xf = x.reshape(B * S, H)
    in_maps = []
    for c in range(NCORES):
        xtc, xbc = _pack_x(xf[c * TPC:(c + 1) * TPC])
        in_maps.append({"xt": xtc, "xb8": xbc, "az": az, "wp": wp8,
                        "bpk2": bpk2, "e": e_np, "cen": cen,
                        "bias": biases[c]})

    kwargs = {}
    if TRACE:
        kwargs = dict(trace=True, tmpdir=TRACE_DIR)
    res = bass_utils.run_bass_kernel_spmd(
        nc, in_maps, core_ids=list(range(NCORES)), **kwargs)
    LAST_RESULT = res

    out = np.empty((B * S, H), np.float32)
    for c in range(NCORES):
        out[c * TPC:(c + 1) * TPC] = _unpack_y(res.results[c]["yt"])
    return out.reshape(B, S, H)


# revision 9
# speedup vs baseline: 1.0857x; 1.0857x over previous
"""Trainium2 Bass kernel for NeuroplasticLlama block-sparse adapter (moe_routing).

Contract: kernel(**inputs) takes FULL unsharded inputs (as produced by
setup_inputs) and returns the FULL [4, 4096, 4096] float32 output.

Strategy (data/sequence parallel over 8 cores, 2048 tokens each):
  - Each core's 2048 contiguous tokens belong to exactly one batch, so the
    task embedding contributes only per-core constant bias vectors
    (te @ A folded into the z bias, te @ Wp folded into the coords bias)
    -- h = x + te is never materialized.
  - Routing is rank-3: scores s[t,n] = coords[t]·mu_n - |mu_n|^2/2 with
    coords = x @ Wp + (te @ Wp + bp).  coords is a K=4096 fp8-DoubleRow
    matmul with M=3; scores are then tiny K=4 matmuls producing s token-major
    [t, n] directly (no score transposes).
  - top-3 selection via threshold = 3rd max (MAX8 + mask), gates
    g = exp(s - max) * (s >= thr3) / sum(...)  (DVE chain).
  - z (all 512 block-rank pairs) = x @ A_all, dense fp8 DoubleRow.
    zg = (z/8) * expand4(g) in fp8; delta = block-diag(8*Bm) matmuls run
    4-way ROW-TILED (K=32 strips at partition bases 0/32/64/96 with
    zero-padded weights) so 4 hidden-chunk matmuls stream concurrently in
    one PE pass.  Delta rounds are interleaved between z/coords
    half-groups so the PE never waits on PSUM drains.
  - y = x + delta: psum drained into the x tiles by a balanced mix of
    DVE/Pool direct-psum adds and ACT-copy + bf16 adds, then stored per
    finished [128, 4096] slice (1 MB DMAs).
  - I/O is bf16 (host converts); x also ships as a packed fp8 copy for the
    PE.  Large DMAs: 2 MB fp8 + 4 MB bf16 per macrotile.
"""

import sys

if "/opt/trn_rl_repo" not in sys.path:
    sys.path.insert(0, "/opt/trn_rl_repo")

import numpy as np
import ml_dtypes

H = 4096
NB = 128
BLK = 32
R = 4
B = 4
S = 4096
NCORES = 8
TPC = (B * S) // NCORES  # tokens per core = 2048
T = 512                  # tokens per macrotile
NMT = TPC // T           # 4 macrotiles per core
NKT = H // 128           # 32 k-tiles over the hidden dim
MTW = 8 * 2048           # columns per macrotile in the packed layout
ZSC = 0.125              # z is scaled by 1/8 before fp8, Bm by 8

TRACE = False            # set by test.py for profiling runs
TRACE_DIR = None
LAST_RESULT = None       # BassKernelResults of the last run

_COMPILED = None


def _build():
    import concourse.bacc as bacc
    import concourse.tile as tile
    from concourse import mybir, masks

    f32 = mybir.dt.float32
    bf16 = mybir.dt.bfloat16
    f8 = mybir.dt.float8e4
    AF = mybir.ActivationFunctionType
    AL = mybir.AluOpType
    DR = mybir.MatmulPerfMode.DoubleRow

    nc = bacc.Bacc("TRN2", target_bir_lowering=False, debug=False,
                   num_devices=NCORES)

    xt_d = nc.dram_tensor("xt", [128, NMT * MTW], bf16, kind="ExternalInput")
    xb_d = nc.dram_tensor("xb8", [128, NMT * MTW], f8, kind="ExternalInput")
    az_d = nc.dram_tensor("az", [128, 4 * NKT * 128], f8, kind="ExternalInput")
    wp_d = nc.dram_tensor("wp", [128, NKT * 16], f8, kind="ExternalInput")
    bpk_d = nc.dram_tensor("bpk2", [128, 1024], f8, kind="ExternalInput")
    e_d = nc.dram_tensor("e", [128, 512], bf16, kind="ExternalInput")
    cen_d = nc.dram_tensor("cen", [4, 128], bf16, kind="ExternalInput")
    bias_d = nc.dram_tensor("bias", [128, 5], f32, kind="ExternalInput")
    yt_d = nc.dram_tensor("yt", [128, NMT * MTW], bf16, kind="ExternalOutput")

    xt_ap = xt_d.ap()
    xb_ap = xb_d.ap()
    yt_ap = yt_d.ap()

    with tile.TileContext(nc) as tc:
        from contextlib import ExitStack
        with ExitStack() as ctx:
            cpool = ctx.enter_context(tc.tile_pool(name="consts", bufs=1))
            xpool = ctx.enter_context(tc.tile_pool(name="xg", bufs=3))
            xbpool = ctx.enter_context(tc.tile_pool(name="xb", bufs=2))
            zpool = ctx.enter_context(tc.tile_pool(name="zb", bufs=6))
            gpool = ctx.enter_context(tc.tile_pool(name="gate", bufs=3))
            spool = ctx.enter_context(tc.tile_pool(name="scal", bufs=4))
            pp = ctx.enter_context(tc.tile_pool(name="ps", bufs=2, space="PSUM"))

            NTS = T // 128  # token sub-tiles per macrotile

            # ---- x tiles; mt0 fp8 halves issued before heavy consts ----
            XB, XG, XGV = [], [], []
            for mt in range(NMT):
                xb = xbpool.tile([128, MTW], f8, name="xb", tag="xb")
                xg = xpool.tile([128, MTW], bf16, name="xg", tag="xg")
                XB.append(xb)
                XG.append(xg)
                XGV.append(xg[:].rearrange(
                    "p (g twoc r t) -> p g twoc r t", g=8, twoc=2, r=2))
            # SWDGE (gpsimd) DMA is descriptor-gen bound (~25-60 GB/s on
            # per-partition-row transfers) -- everything goes on the two
            # HWDGE rings: sync = fp8 x + consts + az + stores, scalar = bf16.
            nc.sync.dma_start(XB[0][:, 0:MTW // 2], xb_ap[:, 0:MTW // 2])
            nc.sync.dma_start(XB[0][:, MTW // 2:], xb_ap[:, MTW // 2:MTW])

            wp8 = cpool.tile([128, NKT * 16], f8, name="wp8", tag="wp8")
            nc.sync.dma_start(wp8[:], wp_d.ap()[:])
            cen = cpool.tile([4, 128], bf16, name="cen", tag="cen")
            nc.sync.dma_start(cen[:], cen_d.ap()[:])
            bias = cpool.tile([128, 5], f32, name="bias", tag="bias")
            nc.sync.dma_start(bias[:], bias_d.ap()[:])
            az = []
            for q in range(4):
                t_az = cpool.tile([128, NKT * 128], f8, name=f"az{q}",
                                  tag=f"az{q}")
                nc.sync.dma_start(
                    t_az[:], az_d.ap()[:, q * NKT * 128:(q + 1) * NKT * 128])
                az.append(t_az)
            esb = cpool.tile([128, 512], bf16, name="esb", tag="esb")
            nc.sync.dma_start(esb[:], e_d.ap()[:])
            bpk2 = cpool.tile([128, 1024], f8, name="bpk2", tag="bpk2")
            nc.sync.dma_start(bpk2[:], bpk_d.ap()[:])
            identf = cpool.tile([128, 128], f32, name="identf", tag="identf")
            masks.make_identity(nc, identf[:])

            # bf16 x on the scalar HWDGE ring (parallel with sync's ring)
            nc.scalar.dma_start(XG[0][:], xt_ap[:, 0:MTW])
            for mt in range(1, NMT):
                nc.sync.dma_start(XB[mt][:],
                                  xb_ap[:, mt * MTW:(mt + 1) * MTW])
                nc.scalar.dma_start(XG[mt][:],
                                    xt_ap[:, mt * MTW:(mt + 1) * MTW])

            ZB = [[None] * 4 for _ in range(NMT)]
            ZG = [[None] * 4 for _ in range(NMT)]
            GT = [None] * NMT
            GGs = [None] * NMT
            CSB = [None] * NMT

            # ---------------- stage helpers ----------------
            def coords_half(mt, cp, lo, hi):
                for k2 in range(lo, hi):
                    nc.tensor.matmul(
                        cp[:],
                        wp8[:, k2 * 32:(k2 + 1) * 32]
                        .rearrange("p (two m) -> p two m", two=2),
                        XB[mt][:, k2 * 2 * T:(k2 + 1) * 2 * T]
                        .rearrange("p (two t) -> p two t", two=2),
                        start=(k2 == 0), stop=(k2 == NKT // 2 - 1),
                        perf_mode=DR,
                    )

            def csb_stage(mt, cp):
                csb = gpool.tile([4, T], bf16, name="csb", tag="csb", bufs=2)
                nc.gpsimd.memset(csb[:], 1.0)
                nc.scalar.activation(csb[0:3, :], cp[0:3, :], AF.Identity,
                                     bias=bias[0:3, 4:5], scale=1.0)
                CSB[mt] = csb

            def scores_stage(mt):
                csb = CSB[mt]
                sp = pp.tile([128, 4 * 128], f32, space="PSUM", name="sp",
                             tag="sp", bufs=1)
                for ts in range(NTS):
                    nc.tensor.matmul(sp[:, ts * 128:(ts + 1) * 128],
                                     csb[:, ts * 128:(ts + 1) * 128],
                                     cen[:], start=True, stop=True)
                ggs = []
                for ts in range(NTS):
                    ssl = sp[:, ts * 128:(ts + 1) * 128]
                    m8 = spool.tile([128, 8], f32, name="m8", tag="m8")
                    nc.vector.max(m8[:], ssl)
                    nr1 = spool.tile([128, 1], f32, name="nr1", tag="nr1")
                    nc.vector.tensor_scalar_mul(nr1[:], m8[:, 0:1], -1.0)
                    ex = gpool.tile([128, 128], f32, name="ex", tag="ex")
                    nc.scalar.activation(ex[:], ssl, AF.Exp, bias=nr1[:],
                                         scale=1.0)
                    em = gpool.tile([128, 128], f32, name="em", tag="em")
                    zs = spool.tile([128, 1], f32, name="zs", tag="zs")
                    nc.vector.scalar_tensor_tensor(em[:], ssl, m8[:, 2:3],
                                                   ex[:], AL.is_ge, AL.mult,
                                                   accum_out=zs[:])
                    rz = spool.tile([128, 1], f32, name="rz", tag="rz")
                    nc.vector.reciprocal(rz[:], zs[:])
                    gg = gpool.tile([128, 128], f32, name="gg", tag="gg",
                                    bufs=NTS + 1)
                    nc.gpsimd.tensor_scalar_mul(gg[:], em[:], rz[:])
                    ggs.append(gg)
                GGs[mt] = ggs

            def z_half(mt, q, zp, lo, hi):
                for k2 in range(lo, hi):
                    nc.tensor.matmul(
                        zp[:],
                        az[q][:, k2 * 256:(k2 + 1) * 256]
                        .rearrange("p (two m) -> p two m", two=2),
                        XB[mt][:, k2 * 2 * T:(k2 + 1) * 2 * T]
                        .rearrange("p (two t) -> p two t", two=2),
                        start=(k2 == 0), stop=(k2 == NKT // 2 - 1),
                        perf_mode=DR,
                    )

            def zb_stage(mt, q, zp):
                zb = zpool.tile([128, T], bf16, name="zb", tag="zb")
                nc.scalar.activation(zb[:], zp[:], AF.Identity,
                                     bias=bias[:, q:q + 1], scale=ZSC)
                ZB[mt][q] = zb

            def transpose_stage(mt):
                gt_sb = gpool.tile([128, T], bf16, name="gt_sb", tag="gt_sb",
                                   bufs=2)
                g_ps = pp.tile([128, 4 * 128], f32, space="PSUM", name="g_ps",
                               tag="sp", bufs=1)
                for ts in range(NTS):
                    nc.tensor.transpose(g_ps[:, ts * 128:(ts + 1) * 128],
                                        GGs[mt][ts][:], identf[:])
                nc.scalar.copy(gt_sb[:], g_ps[:])
                GT[mt] = gt_sb

            def expand_stage(mt, q):
                gx = pp.tile([128, T], f32, space="PSUM", name="gx",
                             tag="zp", bufs=2)
                nc.tensor.matmul(gx[:],
                                 esb[:, q * 128:(q + 1) * 128],
                                 GT[mt][:],
                                 start=True, stop=True)
                zg = zpool.tile([128, T], f8, name="zg", tag="zg")
                nc.vector.tensor_mul(zg[:], ZB[mt][q][:], gx[:])
                ZG[mt][q] = zg

            drain_ctr = [0]

            def dq_round(mt, q, r, last=False):
                zg = ZG[mt][q]
                dp = pp.tile([128, 2048], f32, space="PSUM", name="dp",
                             tag="dp", bufs=1)
                for s in range(4):
                    nc.tensor.matmul(
                        dp[:, s * T:(s + 1) * T],
                        bpk2[32 * s:32 * s + 32,
                             (q * 2 + r) * 128:(q * 2 + r + 1) * 128],
                        zg[32 * s:32 * s + 32, :],
                        start=True, stop=True,
                        tile_position=(32 * s, 0))
                i = drain_ctr[0]
                drain_ctr[0] += 1
                # GPSIMD cannot read PSUM, and its tensor ops are ~4x slower
                # than DVE: psum is drained by DVE (direct [128,2048] add) or
                # ACT (copy to bf16) + a DVE/Pool bf16 add.
                dst4 = XGV[mt][:, 2 * q:2 * q + 2, :, r, :]
                if last:
                    # split across DVE + ACT for low dp-recycle latency
                    d1 = XGV[mt][:, 2 * q, :, r, :]
                    s1 = dp[:, 0:1024].rearrange("p (two t) -> p two t",
                                                 two=2)
                    nc.vector.tensor_add(d1, d1, s1)
                    dsb = zpool.tile([128, 1024], bf16, name="dsbl",
                                     tag="dsbl", bufs=2)
                    nc.scalar.copy(dsb[:], dp[:, 1024:2048])
                    d2 = XGV[mt][:, 2 * q + 1, :, r, :]
                    nc.vector.tensor_add(
                        d2, d2,
                        dsb[:].rearrange("p (two t) -> p two t", two=2))
                elif i % 8 in (0, 3, 6):
                    src4 = dp[:].rearrange("p (g twoc t) -> p g twoc t",
                                           g=2, twoc=2)
                    nc.vector.tensor_add(dst4, dst4, src4)
                else:
                    dsb = zpool.tile([128, 2048], bf16, name="dsb",
                                     tag="dsb", bufs=4)
                    nc.scalar.copy(dsb[:], dp[:])
                    dv4 = dsb[:].rearrange("p (g twoc t) -> p g twoc t",
                                           g=2, twoc=2)
                    if i % 3 == 2:
                        nc.gpsimd.tensor_add(dst4, dst4, dv4)
                    else:
                        nc.vector.tensor_add(dst4, dst4, dv4)

            def dq_store(mt, q, last=False):
                nc.sync.dma_start(
                    yt_ap[:, mt * MTW + q * 4096:mt * MTW + (q + 1) * 4096],
                    XG[mt][:, q * 4096:(q + 1) * 4096])

            # ---------------- interleaved emission ----------------
            for mt in range(NMT):
                p = mt - 1
                cp = pp.tile([16, T], f32, space="PSUM", name="cp", tag="cp",
                             bufs=1)
                coords_half(mt, cp, 0, NKT // 4)
                if p >= 0:
                    dq_round(p, 2, 0)
                coords_half(mt, cp, NKT // 4, NKT // 2)
                csb_stage(mt, cp)
                if p >= 0:
                    dq_round(p, 2, 1)
                    dq_store(p, 2)
                scores_stage(mt)

                zp0 = pp.tile([128, T], f32, space="PSUM", name="zp",
                              tag="zp")
                z_half(mt, 0, zp0, 0, NKT // 4)
                if p >= 0:
                    dq_round(p, 3, 0)
                z_half(mt, 0, zp0, NKT // 4, NKT // 2)
                if p >= 0:
                    dq_round(p, 3, 1)
                    dq_store(p, 3)
                zb_stage(mt, 0, zp0)

                zp1 = pp.tile([128, T], f32, space="PSUM", name="zp",
                              tag="zp")
                z_half(mt, 1, zp1, 0, NKT // 4)
                transpose_stage(mt)
                z_half(mt, 1, zp1, NKT // 4, NKT // 2)
                zb_stage(mt, 1, zp1)

                expand_stage(mt, 0)

                zp2 = pp.tile([128, T], f32, space="PSUM", name="zp",
                              tag="zp")
                z_half(mt, 2, zp2, 0, NKT // 4)
                dq_round(mt, 0, 0)
                expand_stage(mt, 1)
                z_half(mt, 2, zp2, NKT // 4, NKT // 2)
                dq_round(mt, 0, 1)
                dq_store(mt, 0)
                zb_stage(mt, 2, zp2)

                expand_stage(mt, 2)

                zp3 = pp.tile([128, T], f32, space="PSUM", name="zp",
                              tag="zp")
                z_half(mt, 3, zp3, 0, NKT // 4)
                dq_round(mt, 1, 0)
                z_half(mt, 3, zp3, NKT // 4, NKT // 2)
                dq_round(mt, 1, 1)
                dq_store(mt, 1)
                zb_stage(mt, 3, zp3)

                expand_stage(mt, 3)

            # tail: last macrotile's q2/q3 deltas
            m3 = NMT - 1
            dq_round(m3, 2, 0, last=True)
            dq_round(m3, 2, 1, last=True)
            dq_store(m3, 2)
            dq_round(m3, 3, 0, last=True)
            dq_round(m3, 3, 1, last=True)
            dq_store(m3, 3, last=True)

    nc.compile()
    return nc


def _prep_consts(task_emb, task_ids, Wp, bp, centers, A, Bm, adapter_scale):
    scale = float(np.asarray(adapter_scale))
    A_all = np.ascontiguousarray(
        A.transpose(1, 0, 2).reshape(H, NB * R).astype(np.float32))

    # az: [p, q, hc, m] = A_all[hc*128+p, q*128+m], fp8 e4m3 (DoubleRow pairs
    # of consecutive hc become the [K,2,M] interleave)
    az = (A_all.reshape(NKT, 128, 4, 128).transpose(1, 2, 0, 3)
          .reshape(128, 4 * NKT * 128).astype(ml_dtypes.float8_e4m3))
    az = np.ascontiguousarray(az)

    # wp8: [p, k2, two, c] = Wp[(2*k2+two)*128+p, c] (c padded 3->16:
    # fp8 DoubleRow LDWEIGHTS requires the pair stride to be 16B-aligned), fp8
    wpp = np.zeros((H, 16), np.float32)
    wpp[:, 0:3] = Wp.astype(np.float32)
    wp8 = (wpp.reshape(NKT // 2, 2, 128, 16)
           .transpose(2, 0, 1, 3).reshape(128, NKT * 16)
           .astype(ml_dtypes.float8_e4m3))
    wp8 = np.ascontiguousarray(wp8)

    # block-diag up-projection, row-tiled layout (x8 to keep fp8 in normal
    # range; z is /8).  bpk2[32*s + 16*r + mblk*4 + rr,
    #                       (q*2+r)*128 + mblk*32 + c] = Bm[n, rr, c]*scale*8
    # with chunk j = 2*s + r of q, n = (8*q + j)*4 + mblk; other rows zero.
    bpk2 = np.zeros((128, 1024), np.float32)
    for q in range(4):
        for s in range(4):
            for r in range(2):
                j = 2 * s + r
                hc = 8 * q + j
                for mblk in range(4):
                    n = hc * 4 + mblk
                    for rr in range(R):
                        row = 32 * s + 16 * r + mblk * 4 + rr
                        col = (q * 2 + r) * 128 + mblk * 32
                        bpk2[row, col:col + 32] = Bm[n, rr, :] * scale * 8.0
    bpk2 = bpk2.astype(ml_dtypes.float8_e4m3)

    e_np = (np.arange(128)[:, None] == (np.arange(512)[None, :] // 4)) \
        .astype(ml_dtypes.bfloat16)

    # cen_aug: rows 0-2 = centers.T, row 3 = -|mu|^2/2
    cen = np.zeros((4, 128), np.float32)
    cen[0:3] = centers.T
    cen[3] = -0.5 * (centers ** 2).sum(-1)
    cen = np.ascontiguousarray(cen.astype(ml_dtypes.bfloat16))

    biases = []
    for c in range(NCORES):
        te = task_emb[int(np.asarray(task_ids)[c // 2])].astype(np.float32)
        b5 = np.zeros((128, 5), np.float32)
        zoff = (te @ A_all) * ZSC                                # [512]
        for q in range(4):
            b5[:, q] = zoff[q * 128:(q + 1) * 128]
        b5[0:3, 4] = te @ Wp + bp                                # coords bias
        biases.append(np.ascontiguousarray(b5))
    return az, wp8, bpk2, e_np, cen, biases


def _pack_x(xc):
    # [TPC, H] f32 -> [128, NMT*8*2048] in kernel tile order (bf16 + fp8)
    t = np.ascontiguousarray(xc.reshape(NMT, T, 8, 4, 128)
                             .transpose(4, 0, 2, 3, 1)
                             .reshape(128, NMT * MTW))
    return t.astype(ml_dtypes.bfloat16), t.astype(ml_dtypes.float8_e4m3)


def _unpack_y(yt):
    # [128, NMT*8*2048] bf16 -> [TPC, H] f32
    t = yt.reshape(128, NMT, 8, 4, T).transpose(1, 4, 2, 3, 0)
    return t.reshape(TPC, H).astype(np.float32)


def kernel(x, task_ids, task_emb, Wp, bp, centers, A, Bm, adapter_scale):
    global _COMPILED, LAST_RESULT
    from concourse import bass_utils

    x = np.asarray(x, dtype=np.float32)
    task_ids = np.asarray(task_ids)
    task_emb = np.asarray(task_emb, dtype=np.float32)
    Wp = np.asarray(Wp, dtype=np.float32)
    bp = np.asarray(bp, dtype=np.float32)
    centers = np.asarray(centers, dtype=np.float32)
    A = np.asarray(A, dtype=np.float32)
    Bm = np.asarray(Bm, dtype=np.float32)

    if _COMPILED is None:
        _COMPILED = _build()
    nc = _COMPILED

    az, wp8, bpk2, e_np, cen, biases = _prep_consts(
        task_emb, task_ids, Wp, bp, centers, A, Bm, adapter_scale)

    xf = x.reshape(B * S, H)
    in_maps = []
    for c in range(NCORES):
        xtc, xbc = _pack_x(xf[c * TPC:(c + 1) * TPC])
        in_maps.append({"xt": xtc, "xb8": xbc, "az": az, "wp": wp8,
                        "bpk2": bpk2, "e": e_np, "cen": cen,
                        "bias": biases[c]})

    kwargs = {}
    if TRACE:
        kwargs = dict(trace=True, tmpdir=TRACE_DIR)
    res = bass_utils.run_bass_kernel_spmd(
        nc, in_maps, core_ids=list(range(NCORES)), **kwargs)
    LAST_RESULT = res

    out = np.empty((B * S, H), np.float32)
    for c in range(NCORES):
        out[c * TPC:(c + 1) * TPC] = _unpack_y(res.results[c]["yt"])
    return out.reshape(B, S, H)


# revision 12
# speedup vs baseline: 1.1647x; 1.0728x over previous
"""Trainium2 Bass kernel for NeuroplasticLlama block-sparse adapter (moe_routing).

Contract: kernel(**inputs) takes FULL unsharded inputs (as produced by
setup_inputs) and returns the FULL [4, 4096, 4096] float32 output.

Strategy (data/sequence parallel over 8 cores, 2048 tokens each):
  - Each core's 2048 contiguous tokens belong to exactly one batch, so the
    task embedding contributes only per-core constant bias vectors
    (te @ A folded into the z bias, te @ Wp folded into the coords bias)
    -- h = x + te is never materialized.
  - Routing is rank-3: scores s[t,n] = coords[t]·mu_n - |mu_n|^2/2 with
    coords = x @ Wp + (te @ Wp + bp).  coords is a K=4096 fp8-DoubleRow
    matmul with M=3; scores are then tiny K=4 matmuls producing s token-major
    [t, n] directly (no score transposes).
  - top-3 selection via threshold = 3rd max (MAX8 + mask), gates
    g = exp(s - max) * (s >= thr3) / sum(...)  (DVE chain).
  - z (all 512 block-rank pairs) = x @ A_all, dense fp8 DoubleRow.
    zg = (z/8) * expand4(g) in fp8; delta = block-diag(8*Bm) matmuls run
    4-way ROW-TILED (K=32 strips at partition bases 0/32/64/96 with
    zero-padded weights) so 4 hidden-chunk matmuls stream concurrently in
    one PE pass.  Delta rounds are interleaved between z/coords
    half-groups so the PE never waits on PSUM drains.
  - y = x + delta: psum drained into the x tiles by a balanced mix of
    DVE/Pool direct-psum adds and ACT-copy + bf16 adds, then stored per
    finished [128, 4096] slice (1 MB DMAs).
  - I/O is bf16 (host converts); x also ships as a packed fp8 copy for the
    PE.  Large DMAs: 2 MB fp8 + 4 MB bf16 per macrotile.
"""

import sys

if "/opt/trn_rl_repo" not in sys.path:
    sys.path.insert(0, "/opt/trn_rl_repo")

import numpy as np
import ml_dtypes

H = 4096
NB = 128
BLK = 32
R = 4
B = 4
S = 4096
NCORES = 8
TPC = (B * S) // NCORES  # tokens per core = 2048
T = 512                  # tokens per macrotile
NMT = TPC // T           # 4 macrotiles per core
NKT = H // 128           # 32 k-tiles over the hidden dim
MTW = 8 * 2048           # columns per macrotile in the packed layout
ZSC = 0.125              # z is scaled by 1/8 before fp8, Bm by 8

TRACE = False            # set by test.py for profiling runs
TRACE_DIR = None
LAST_RESULT = None       # BassKernelResults of the last run

_COMPILED = None


def _build():
    import concourse.bacc as bacc
    import concourse.tile as tile
    from concourse import mybir, masks

    f32 = mybir.dt.float32
    bf16 = mybir.dt.bfloat16
    f8 = mybir.dt.float8e4
    AF = mybir.ActivationFunctionType
    AL = mybir.AluOpType
    DR = mybir.MatmulPerfMode.DoubleRow

    nc = bacc.Bacc("TRN2", target_bir_lowering=False, debug=False,
                   num_devices=NCORES)

    xt_d = nc.dram_tensor("xt", [128, NMT * MTW], bf16, kind="ExternalInput")
    xb_d = nc.dram_tensor("xb8", [128, NMT * MTW], f8, kind="ExternalInput")
    az_d = nc.dram_tensor("az", [128, 4 * NKT * 128], f8, kind="ExternalInput")
    wp_d = nc.dram_tensor("wp", [128, NKT * 16], f8, kind="ExternalInput")
    bpk_d = nc.dram_tensor("bpk2", [128, 1024], f8, kind="ExternalInput")
    e_d = nc.dram_tensor("e", [128, 512], bf16, kind="ExternalInput")
    cen_d = nc.dram_tensor("cen", [4, 128], bf16, kind="ExternalInput")
    bias_d = nc.dram_tensor("bias", [128, 5], f32, kind="ExternalInput")
    yt_d = nc.dram_tensor("yt", [128, NMT * MTW], bf16, kind="ExternalOutput")

    xt_ap = xt_d.ap()
    xb_ap = xb_d.ap()
    yt_ap = yt_d.ap()

    with tile.TileContext(nc) as tc:
        from contextlib import ExitStack
        with ExitStack() as ctx:
            cpool = ctx.enter_context(tc.tile_pool(name="consts", bufs=1))
            xpool = ctx.enter_context(tc.tile_pool(name="xg", bufs=3))
            xbpool = ctx.enter_context(tc.tile_pool(name="xb", bufs=2))
            zpool = ctx.enter_context(tc.tile_pool(name="zb", bufs=6))
            gpool = ctx.enter_context(tc.tile_pool(name="gate", bufs=3))
            spool = ctx.enter_context(tc.tile_pool(name="scal", bufs=4))
            pp = ctx.enter_context(tc.tile_pool(name="ps", bufs=2, space="PSUM"))

            NTS = T // 128  # token sub-tiles per macrotile

            # ---- x tiles; mt0 fp8 halves issued before heavy consts ----
            XB, XG, XGV = [], [], []
            for mt in range(NMT):
                xb = xbpool.tile([128, MTW], f8, name="xb", tag="xb")
                xg = xpool.tile([128, MTW], bf16, name="xg", tag="xg")
                XB.append(xb)
                XG.append(xg)
                XGV.append(xg[:].rearrange(
                    "p (g twoc r t) -> p g twoc r t", g=8, twoc=2, r=2))
            # SWDGE (gpsimd) DMA is descriptor-gen bound (~25-60 GB/s on
            # per-partition-row transfers) -- everything goes on the two
            # HWDGE rings in need order: sync = fp8 x + consts + az + stores,
            # scalar = bf16 x (its issues are deferred into the mt pipeline
            # so they don't steal HBM bandwidth from the critical-path ring).
            nc.sync.dma_start(XB[0][:, 0:MTW // 2], xb_ap[:, 0:MTW // 2])
            wp8 = cpool.tile([128, NKT * 16], f8, name="wp8", tag="wp8")
            nc.sync.dma_start(wp8[:], wp_d.ap()[:])
            cen = cpool.tile([4, 128], bf16, name="cen", tag="cen")
            nc.sync.dma_start(cen[:], cen_d.ap()[:])
            bias = cpool.tile([128, 5], f32, name="bias", tag="bias")
            nc.sync.dma_start(bias[:], bias_d.ap()[:])
            az = []
            az_d_ap = az_d.ap()

            def load_az(q):
                t_az = cpool.tile([128, NKT * 128], f8, name=f"az{q}",
                                  tag=f"az{q}")
                nc.sync.dma_start(
                    t_az[:], az_d_ap[:, q * NKT * 128:(q + 1) * NKT * 128])
                az.append(t_az)

            load_az(0)
            nc.sync.dma_start(XB[0][:, MTW // 2:], xb_ap[:, MTW // 2:MTW])
            nc.scalar.dma_start(XG[0][:, 0:MTW // 2], xt_ap[:, 0:MTW // 2])
            for q in range(1, 4):
                load_az(q)
            esb = cpool.tile([128, 512], bf16, name="esb", tag="esb")
            nc.sync.dma_start(esb[:], e_d.ap()[:])
            bpk2 = cpool.tile([128, 1024], f8, name="bpk2", tag="bpk2")
            nc.sync.dma_start(bpk2[:], bpk_d.ap()[:])
            identf = cpool.tile([128, 128], f32, name="identf", tag="identf")
            masks.make_identity(nc, identf[:])

            for mt in range(1, NMT):
                nc.sync.dma_start(XB[mt][:],
                                  xb_ap[:, mt * MTW:(mt + 1) * MTW])

            # deferred bf16 x issues: (engine-emission point, slices)
            XT_PENDING = {
                0: [(XG[0][:, MTW // 2:], xt_ap[:, MTW // 2:MTW]),
                    (XG[1][:], xt_ap[:, MTW:2 * MTW])],
                1: [(XG[2][:], xt_ap[:, 2 * MTW:3 * MTW])],
                2: [(XG[3][:], xt_ap[:, 3 * MTW:4 * MTW])],
            }

            ZB = [[None] * 4 for _ in range(NMT)]
            ZG = [[None] * 4 for _ in range(NMT)]
            GT = [None] * NMT
            GGs = [None] * NMT
            CSB = [None] * NMT

            # ---------------- stage helpers ----------------
            def coords_half(mt, cp, lo, hi):
                for k2 in range(lo, hi):
                    nc.tensor.matmul(
                        cp[:],
                        wp8[:, k2 * 32:(k2 + 1) * 32]
                        .rearrange("p (two m) -> p two m", two=2),
                        XB[mt][:, k2 * 2 * T:(k2 + 1) * 2 * T]
                        .rearrange("p (two t) -> p two t", two=2),
                        start=(k2 == 0), stop=(k2 == NKT // 2 - 1),
                        perf_mode=DR,
                    )

            def csb_stage(mt, cp):
                csb = gpool.tile([4, T], bf16, name="csb", tag="csb", bufs=2)
                nc.gpsimd.memset(csb[:], 1.0)
                nc.scalar.activation(csb[0:3, :], cp[0:3, :], AF.Identity,
                                     bias=bias[0:3, 4:5], scale=1.0)
                for dst, src in XT_PENDING.pop(mt, []):
                    nc.scalar.dma_start(dst, src)
                CSB[mt] = csb

            def scores_stage(mt):
                csb = CSB[mt]
                sp = pp.tile([128, 4 * 128], f32, space="PSUM", name="sp",
                             tag="sp", bufs=1)
                for ts in range(NTS):
                    nc.tensor.matmul(sp[:, ts * 128:(ts + 1) * 128],
                                     csb[:, ts * 128:(ts + 1) * 128],
                                     cen[:], start=True, stop=True)
                ggs = []
                for ts in range(NTS):
                    ssl = sp[:, ts * 128:(ts + 1) * 128]
                    m8 = spool.tile([128, 8], f32, name="m8", tag="m8")
                    nc.vector.max(m8[:], ssl)
                    nr1 = spool.tile([128, 1], f32, name="nr1", tag="nr1")
                    nc.vector.tensor_scalar_mul(nr1[:], m8[:, 0:1], -1.0)
                    ex = gpool.tile([128, 128], f32, name="ex", tag="ex")
                    nc.scalar.activation(ex[:], ssl, AF.Exp, bias=nr1[:],
                                         scale=1.0)
                    em = gpool.tile([128, 128], f32, name="em", tag="em")
                    zs = spool.tile([128, 1], f32, name="zs", tag="zs")
                    nc.vector.scalar_tensor_tensor(em[:], ssl, m8[:, 2:3],
                                                   ex[:], AL.is_ge, AL.mult,
                                                   accum_out=zs[:])
                    rz = spool.tile([128, 1], f32, name="rz", tag="rz")
                    nc.vector.reciprocal(rz[:], zs[:])
                    gg = gpool.tile([128, 128], f32, name="gg", tag="gg",
                                    bufs=NTS + 1)
                    nc.gpsimd.tensor_scalar_mul(gg[:], em[:], rz[:])
                    ggs.append(gg)
                GGs[mt] = ggs

            def z_half(mt, q, zp, lo, hi):
                for k2 in range(lo, hi):
                    nc.tensor.matmul(
                        zp[:],
                        az[q][:, k2 * 256:(k2 + 1) * 256]
                        .rearrange("p (two m) -> p two m", two=2),
                        XB[mt][:, k2 * 2 * T:(k2 + 1) * 2 * T]
                        .rearrange("p (two t) -> p two t", two=2),
                        start=(k2 == 0), stop=(k2 == NKT // 2 - 1),
                        perf_mode=DR,
                    )

            def zb_stage(mt, q, zp):
                zb = zpool.tile([128, T], bf16, name="zb", tag="zb")
                nc.scalar.activation(zb[:], zp[:], AF.Identity,
                                     bias=bias[:, q:q + 1], scale=ZSC)
                ZB[mt][q] = zb

            def transpose_stage(mt):
                gt_sb = gpool.tile([128, T], bf16, name="gt_sb", tag="gt_sb",
                                   bufs=2)
                g_ps = pp.tile([128, 4 * 128], f32, space="PSUM", name="g_ps",
                               tag="sp", bufs=1)
                for ts in range(NTS):
                    nc.tensor.transpose(g_ps[:, ts * 128:(ts + 1) * 128],
                                        GGs[mt][ts][:], identf[:])
                nc.scalar.copy(gt_sb[:], g_ps[:])
                GT[mt] = gt_sb

            def expand_stage(mt, q):
                gx = pp.tile([128, T], f32, space="PSUM", name="gx",
                             tag="zp", bufs=2)
                nc.tensor.matmul(gx[:],
                                 esb[:, q * 128:(q + 1) * 128],
                                 GT[mt][:],
                                 start=True, stop=True)
                zg = zpool.tile([128, T], f8, name="zg", tag="zg")
                nc.vector.tensor_mul(zg[:], ZB[mt][q][:], gx[:])
                ZG[mt][q] = zg

            drain_ctr = [0]

            def dq_round(mt, q, r, last=False):
                zg = ZG[mt][q]
                dp = pp.tile([128, 2048], f32, space="PSUM", name="dp",
                             tag="dp", bufs=1)
                for s in range(4):
                    nc.tensor.matmul(
                        dp[:, s * T:(s + 1) * T],
                        bpk2[32 * s:32 * s + 32,
                             (q * 2 + r) * 128:(q * 2 + r + 1) * 128],
                        zg[32 * s:32 * s + 32, :],
                        start=True, stop=True,
                        tile_position=(32 * s, 0))
                i = drain_ctr[0]
                drain_ctr[0] += 1
                # GPSIMD cannot read PSUM, and its tensor ops are ~4x slower
                # than DVE: psum is drained by DVE (direct [128,2048] add) or
                # ACT (copy to bf16) + a DVE/Pool bf16 add.
                dst4 = XGV[mt][:, 2 * q:2 * q + 2, :, r, :]
                if last:
                    # split across DVE + ACT for low dp-recycle latency
                    d1 = XGV[mt][:, 2 * q, :, r, :]
                    s1 = dp[:, 0:1024].rearrange("p (two t) -> p two t",
                                                 two=2)
                    nc.vector.tensor_add(d1, d1, s1)
                    dsb = zpool.tile([128, 1024], bf16, name="dsbl",
                                     tag="dsbl", bufs=2)
                    nc.scalar.copy(dsb[:], dp[:, 1024:2048])
                    d2 = XGV[mt][:, 2 * q + 1, :, r, :]
                    nc.vector.tensor_add(
                        d2, d2,
                        dsb[:].rearrange("p (two t) -> p two t", two=2))
                elif i % 8 in (0, 3, 6):
                    src4 = dp[:].rearrange("p (g twoc t) -> p g twoc t",
                                           g=2, twoc=2)
                    nc.vector.tensor_add(dst4, dst4, src4)
                else:
                    dsb = zpool.tile([128, 2048], bf16, name="dsb",
                                     tag="dsb", bufs=4)
                    nc.scalar.copy(dsb[:], dp[:])
                    dv4 = dsb[:].rearrange("p (g twoc t) -> p g twoc t",
                                           g=2, twoc=2)
                    if i % 3 == 2:
                        nc.gpsimd.tensor_add(dst4, dst4, dv4)
                    else:
                        nc.vector.tensor_add(dst4, dst4, dv4)

            def dq_store(mt, q, last=False):
                nc.sync.dma_start(
                    yt_ap[:, mt * MTW + q * 4096:mt * MTW + (q + 1) * 4096],
                    XG[mt][:, q * 4096:(q + 1) * 4096])

            # ---------------- interleaved emission ----------------
            for mt in range(NMT):
                p = mt - 1
                cp = pp.tile([16, T], f32, space="PSUM", name="cp", tag="cp",
                             bufs=1)
                coords_half(mt, cp, 0, NKT // 4)
                if p >= 0:
                    dq_round(p, 2, 0)
                coords_half(mt, cp, NKT // 4, NKT // 2)
                csb_stage(mt, cp)
                if p >= 0:
                    dq_round(p, 2, 1)
                    dq_store(p, 2)
                scores_stage(mt)

                zp0 = pp.tile([128, T], f32, space="PSUM", name="zp",
                              tag="zp")
                z_half(mt, 0, zp0, 0, NKT // 4)
                if p >= 0:
                    dq_round(p, 3, 0)
                z_half(mt, 0, zp0, NKT // 4, NKT // 2)
                if p >= 0:
                    dq_round(p, 3, 1)
                    dq_store(p, 3)
                zb_stage(mt, 0, zp0)

                zp1 = pp.tile([128, T], f32, space="PSUM", name="zp",
                              tag="zp")
                z_half(mt, 1, zp1, 0, NKT // 4)
                transpose_stage(mt)
                z_half(mt, 1, zp1, NKT // 4, NKT // 2)
                zb_stage(mt, 1, zp1)

                expand_stage(mt, 0)

                zp2 = pp.tile([128, T], f32, space="PSUM", name="zp",
                              tag="zp")
                z_half(mt, 2, zp2, 0, NKT // 4)
                dq_round(mt, 0, 0)
                expand_stage(mt, 1)
                z_half(mt, 2, zp2, NKT // 4, NKT // 2)
                dq_round(mt, 0, 1)
                dq_store(mt, 0)
                zb_stage(mt, 2, zp2)

                expand_stage(mt, 2)

                zp3 = pp.tile([128, T], f32, space="PSUM", name="zp",
                              tag="zp")
                z_half(mt, 3, zp3, 0, NKT // 4)
                dq_round(mt, 1, 0)
                z_half(mt, 3, zp3, NKT // 4, NKT // 2)
                dq_round(mt, 1, 1)
                dq_store(mt, 1)
                zb_stage(mt, 3, zp3)

                expand_stage(mt, 3)

            # tail: last macrotile's q2/q3 deltas; store each 512 KB g-slice
            # as soon as its two rounds have drained
            m3 = NMT - 1
            for q in (2, 3):
                dq_round(m3, q, 0, last=True)
                dq_round(m3, q, 1, last=True)
                for g2 in range(2):
                    gcol = (2 * q + g2) * 2048
                    nc.sync.dma_start(
                        yt_ap[:, m3 * MTW + gcol:m3 * MTW + gcol + 2048],
                        XG[m3][:, gcol:gcol + 2048])

    nc.compile()
    return nc


def _prep_consts(task_emb, task_ids, Wp, bp, centers, A, Bm, adapter_scale):
    scale = float(np.asarray(adapter_scale))
    A_all = np.ascontiguousarray(
        A.transpose(1, 0, 2).reshape(H, NB * R).astype(np.float32))

    # az: [p, q, hc, m] = A_all[hc*128+p, q*128+m], fp8 e4m3 (DoubleRow pairs
    # of consecutive hc become the [K,2,M] interleave)
    az = (A_all.reshape(NKT, 128, 4, 128).transpose(1, 2, 0, 3)
          .reshape(128, 4 * NKT * 128).astype(ml_dtypes.float8_e4m3))
    az = np.ascontiguousarray(az)

    # wp8: [p, k2, two, c] = Wp[(2*k2+two)*128+p, c] (c padded 3->16:
    # fp8 DoubleRow LDWEIGHTS requires the pair stride to be 16B-aligned), fp8
    wpp = np.zeros((H, 16), np.float32)
    wpp[:, 0:3] = Wp.astype(np.float32)
    wp8 = (wpp.reshape(NKT // 2, 2, 128, 16)
           .transpose(2, 0, 1, 3).reshape(128, NKT * 16)
           .astype(ml_dtypes.float8_e4m3))
    wp8 = np.ascontiguousarray(wp8)

    # block-diag up-projection, row-tiled layout (x8 to keep fp8 in normal
    # range; z is /8).  bpk2[32*s + 16*r + mblk*4 + rr,
    #                       (q*2+r)*128 + mblk*32 + c] = Bm[n, rr, c]*scale*8
    # with chunk j = 2*s + r of q, n = (8*q + j)*4 + mblk; other rows zero.
    bpk2 = np.zeros((128, 1024), np.float32)
    for q in range(4):
        for s in range(4):
            for r in range(2):
                j = 2 * s + r
                hc = 8 * q + j
                for mblk in range(4):
                    n = hc * 4 + mblk
                    for rr in range(R):
                        row = 32 * s + 16 * r + mblk * 4 + rr
                        col = (q * 2 + r) * 128 + mblk * 32
                        bpk2[row, col:col + 32] = Bm[n, rr, :] * scale * 8.0
    bpk2 = bpk2.astype(ml_dtypes.float8_e4m3)

    e_np = (np.arange(128)[:, None] == (np.arange(512)[None, :] // 4)) \
        .astype(ml_dtypes.bfloat16)

    # cen_aug: rows 0-2 = centers.T, row 3 = -|mu|^2/2
    cen = np.zeros((4, 128), np.float32)
    cen[0:3] = centers.T
    cen[3] = -0.5 * (centers ** 2).sum(-1)
    cen = np.ascontiguousarray(cen.astype(ml_dtypes.bfloat16))

    biases = []
    for c in range(NCORES):
        te = task_emb[int(np.asarray(task_ids)[c // 2])].astype(np.float32)
        b5 = np.zeros((128, 5), np.float32)
        zoff = (te @ A_all) * ZSC                                # [512]
        for q in range(4):
            b5[:, q] = zoff[q * 128:(q + 1) * 128]
        b5[0:3, 4] = te @ Wp + bp                                # coords bias
        biases.append(np.ascontiguousarray(b5))
    return az, wp8, bpk2, e_np, cen, biases


def _pack_x(xc):
    # [TPC, H] f32 -> [128, NMT*8*2048] in kernel tile order (bf16 + fp8)
    t = np.ascontiguousarray(xc.reshape(NMT, T, 8, 4, 128)
                             .transpose(4, 0, 2, 3, 1)
                             .reshape(128, NMT * MTW))
    return t.astype(ml_dtypes.bfloat16), t.astype(ml_dtypes.float8_e4m3)


def _unpack_y(yt):
    # [128, NMT*8*2048] bf16 -> [TPC, H] f32
    t = yt.reshape(128, NMT, 8, 4, T).transpose(1, 4, 2, 3, 0)
    return t.reshape(TPC, H).astype(np.float32)


def kernel(x, task_ids, task_emb, Wp, bp, centers, A, Bm, adapter_scale):
    global _COMPILED, LAST_RESULT
    from concourse import bass_utils

    x = np.asarray(x, dtype=np.float32)
    task_ids = np.asarray(task_ids)
    task_emb = np.asarray(task_emb, dtype=np.float32)
    Wp = np.asarray(Wp, dtype=np.float32)
    bp = np.asarray(bp, dtype=np.float32)
    centers = np.asarray(centers, dtype=np.float32)
    A = np.asarray(A, dtype=np.float32)
    Bm = np.asarray(Bm, dtype=np.float32)

    if _COMPILED is None:
        _COMPILED = _build()
    nc = _COMPILED

    az, wp8, bpk2, e_np, cen, biases = _prep_consts(
        task_emb, task_ids, Wp, bp, centers, A, Bm, adapter_scale)

    xf = x.reshape(B * S, H)
    in_maps = []
    for c in range(NCORES):
        xtc, xbc = _pack_x(xf[c * TPC:(c + 1) * TPC])
        in_maps.append({"xt": xtc, "xb8": xbc, "az": az, "wp": wp8,
                        "bpk2": bpk2, "e": e_np, "cen": cen,
                        "bias": biases[c]})

    kwargs = {}
    if TRACE:
        kwargs = dict(trace=True, tmpdir=TRACE_DIR)
    res = bass_utils.run_bass_kernel_spmd(
        nc, in_maps, core_ids=list(range(NCORES)), **kwargs)
    LAST_RESULT = res

    out = np.empty((B * S, H), np.float32)
    for c in range(NCORES):
        out[c * TPC:(c + 1) * TPC] = _unpack_y(res.results[c]["yt"])
    return out.reshape(B, S, H)
